# revision 20
# baseline (speedup 1.0000x reference)
"""Trainium2 Bass kernel for nn_MixedResolutionCNN.

Network (per sample, eval mode):
  high branch: ridgelet conv 3->16 k=15 same-pad (kernel broadcast over in-ch)
               -> relu -> maxpool2 -> 4096 feats
  low branch:  bilinear resize 32->8 -> conv 3->4 k=3 pad1 + bias -> relu
               -> maxpool2 -> 64 feats
  head:        concat -> fc 4160->1024 relu -> 1024->256 relu -> 256->5

Device strategy (pure data parallel over 8 cores, 512 images/core):

* The ridgelet kernel is identical across the 3 input channels, so the high
  conv contracts the channel-summed image xs = sum_c x[:,c] with a 16x15x15
  kernel. Expressed as matmuls with contraction over (v, i') = (kernel col,
  image row): out[(o,i),(b,j)] = sum Khat[o, i'-i+7, v] * xs[b, i', j+v-7].
  The moving operand for v-chunk kc is a skewed 4x replication of the
  column-padded image rows: block dv holds xs shifted by dv columns so a
  single strided AP reads xs[b, i', j + (4kc+dv) - 7] for all 128
  partitions.  4 K-chunks x 4 M-chunks of [128,128,512] matmuls per
  16-image tile.
* relu/maxpool fold into the pool maxes (relu(max(a,b)) == max(0,a,b) via
  one scalar_tensor_tensor op on the vector engine); pooled features are
  written j2-major so every FC1 rhs chunk is a fully contiguous [128,512]
  slab (peak-rate matmul feed).  Channel-sum adds run on gpsimd to keep the
  vector engine under the tensor-engine roofline.
* low branch: resize+conv fold into one linear map [3072, 256]; x is packed
  host-side as [feature, batch] so the 24 K-chunks stream contiguously.
  Output partitions hold the 4 pool-parity groups (2 matmuls of 128).
* FC1 weights (8.5MB bf16) stream from DRAM, double buffered; everything
  else is resident in SBUF.  Conv weights are DMA'd first so the tensor
  engine starts within ~2us.
"""

import numpy as np
import ml_dtypes

import concourse.bass as bass
import concourse.tile as tile
from concourse import mybir
from concourse.alu_op_type import AluOpType
from concourse.bass_utils import run_bass_kernel_spmd

BF16NP = ml_dtypes.bfloat16
FP32 = mybir.dt.float32
BF16 = mybir.dt.bfloat16

B = 4096
NCORES = 8
BC = B // NCORES           # 512 images per core
TIMGS = 16                 # images per tile
NTILES = BC // TIMGS       # 32
KS = 15
OUT_CH = 16


# ---------------------------------------------------------------- host math
def _ridgelet_kernel(r_dirs, r_scales, r_pos):
    """[16,15,15] channel-shared ridgelet kernel, mirrors reference."""
    c = np.arange(KS, dtype=np.float32) - KS // 2
    x1 = c[:, None]
    x2 = c[None, :]
    d = np.asarray(r_dirs, np.float32)[:, None, None]
    s = np.asarray(r_scales, np.float32)[:, None, None]
    p = np.asarray(r_pos, np.float32)[:, None, None]
    t = (x1 * np.cos(d) + x2 * np.sin(d) - p) / s
    vals = np.exp(-t * t / 2.0) - 0.5 * np.exp(-t * t / 8.0)
    return vals.reshape(OUT_CH, 10, KS, KS).sum(axis=1)


def _resize_mat(in_size=32, out_size=8):
    """Row matrix of jax.image.resize(..., 'bilinear', antialias=True)."""
    scale = out_size / in_size
    inv = 1.0 / scale
    kscale = max(inv, 1.0)
    sample_f = (np.arange(out_size, dtype=np.float64) + 0.5) * inv - 0.5
    x = np.abs(sample_f[None, :] - np.arange(in_size, dtype=np.float64)[:, None])
    w = np.maximum(0.0, 1.0 - x / kscale)
    w = w / w.sum(axis=0, keepdims=True)
    return w.T.astype(np.float32)  # [out, in]


def build_weights(inputs):
    """All packed device arrays (shared across cores)."""
    khat = _ridgelet_kernel(inputs["r_dirs"], inputs["r_scales"], inputs["r_pos"])
    # padded to 16x16 so v=15 / u out-of-range index to a zero slot
    khat_p = np.zeros((OUT_CH, 16, 16), np.float32)
    khat_p[:, :KS, :KS] = khat

    # conv lhsT: wc[p=(dv,i'), kc*512 + ch*128 + wi]
    dvip = np.arange(128)
    dv = dvip // 32
    ip = dvip % 32
    m = np.arange(512)
    ch = m // 128
    wi = m % 128
    par = ch // 2          # i parity (0=even rows, 1=odd)
    oh = ch % 2            # o half
    o = oh * 8 + wi // 16
    i2 = wi % 16
    i = 2 * i2 + par
    wc = np.zeros((128, 2048), np.float32)
    u = ip[:, None] - i[None, :] + 7          # [128, 512]
    umask = (u >= 0) & (u < KS)
    uc = np.clip(u, 0, 15)
    for kc in range(4):
        v = 4 * kc + dv                        # [128]
        vals = khat_p[o[None, :], uc, np.clip(v, 0, 15)[:, None]]
        vals = np.where(umask, vals, 0.0)
        wc[:, kc * 512:(kc + 1) * 512] = vals

    # low branch: fold resize+conv into [3072, 256]
    A = _resize_mat()
    Ash = np.zeros((3, 8, 32), np.float32)
    for dh in range(3):
        for ph in range(8):
            r = ph + dh - 1
            if 0 <= r < 8:
                Ash[dh, ph] = A[r]
    wlow = np.asarray(inputs["wlow"], np.float32)
    # D[c,i,w,o,ph,pw] = sum_{dh,dw} wlow[o,c,dh,dw] Ash[dh,ph,i] Ash[dw,pw,w]
    D = np.einsum("ocuv,upi,vqw->ciwopq", wlow, Ash, Ash).astype(np.float32)
    Dp = D.reshape(3072, 4, 8, 8)              # [(c,i,w), o, ph, pw]
    # out col layout: 2 matmuls of 128.  Pool partners sit at the SAME
    # partition in the two PSUM banks (A holds groups 0,2; B holds 1,3) so
    # the first pool max never crosses partitions.
    Wn = np.zeros((3072, 2, 128), np.float32)
    G = [(0, 0), (0, 1), (1, 0), (1, 1)]
    for g, (pp_, qq) in enumerate(G):
        blk = Dp[:, :, pp_::2, qq::2].reshape(3072, 64)
        Wn[:, g % 2, (g // 2) * 64:(g // 2) * 64 + 64] = blk
    wlowp = np.ascontiguousarray(
        Wn.reshape(24, 128, 256).transpose(1, 0, 2).reshape(128, 24 * 256))

    # FC1 reorder: kstep = j2*2 + chunk over high feats, kstep 32 = low
    w1 = np.asarray(inputs["w1"], np.float32)          # [1024, 4160]
    w1hi = w1[:, 64:].reshape(1024, 16, 16, 16)        # [n, o, i2, j2]
    w1r = np.zeros((33, 128, 1024), np.float32)
    for ks in range(32):
        j2, c = ks // 2, ks % 2
        blk = w1hi[:, 8 * c:8 * (c + 1), :, j2]        # [n, 8, 16]
        w1r[ks] = blk.reshape(1024, 128).T
    w1r[32, :64, :] = w1[:, :64].T

    w2 = np.asarray(inputs["w2"], np.float32)          # [256, 1024]
    w2r = np.zeros((128, 2048), np.float32)
    for kc in range(8):
        w2r[:, kc * 256:(kc + 1) * 256] = w2[:, kc * 128:(kc + 1) * 128].T
    w3 = np.asarray(inputs["w3"], np.float32)          # [5, 256]
    w3r = np.zeros((128, 10), np.float32)
    for kc in range(2):
        w3r[:, kc * 5:(kc + 1) * 5] = w3[:, kc * 128:(kc + 1) * 128].T

    b1r = np.asarray(inputs["b1"], np.float32).reshape(8, 128).T.copy()
    b2r = np.asarray(inputs["b2"], np.float32).reshape(2, 128).T.copy()
    b3r = np.asarray(inputs["b3"], np.float32)[:, None].copy()
    blowr = np.repeat(np.asarray(inputs["blow"], np.float32), 16)[:, None].copy()

    return {
        "wc": wc.astype(BF16NP),
        "wlow": wlowp.astype(BF16NP),
        "w1r": w1r.astype(BF16NP),
        "w2r": w2r.astype(BF16NP),
        "w3r": w3r.astype(BF16NP),
        "b1r": np.ascontiguousarray(b1r),
        "b2r": np.ascontiguousarray(b2r),
        "b3r": b3r,
        "blowr": np.ascontiguousarray(blowr),
    }


def pack_x_low(x_core):
    """[512,3,32,32] f32 -> [128, 24*512] bf16: xl[p, kc*512+tb] =
    x[tb, (kc*128+p)//1024, ...] i.e. feature-major transpose."""
    xc = np.asarray(x_core, np.float32).astype(BF16NP)
    arr = xc.transpose(1, 2, 3, 0).reshape(3072, BC)      # [(c,i,w), tb]
    return np.ascontiguousarray(
        arr.reshape(24, 128, BC).transpose(1, 0, 2).reshape(128, 24 * BC))


def pack_x_high(x_core):
    """4x skew-replicated, channel-summed high-branch input.

    xh4[dv*32+i, t*576 + b*36 + jj] = sum_c x[t*16+b, c, i, jj+dv-3]
    (zero outside the image); DMA'd straight into the 48-stride skew
    buffer on device -- no device-side adds at all.
    """
    xs = np.asarray(x_core, np.float32).sum(axis=1).astype(BF16NP)  # [BC,32,32]
    xpad = np.zeros((BC, 32, 42), BF16NP)
    xpad[:, :, 3:35] = xs
    arr = np.stack([xpad[:, :, dv:dv + 36] for dv in range(4)])
    arr = arr.reshape(4, NTILES, TIMGS, 32, 36).transpose(0, 3, 1, 2, 4)
    return np.ascontiguousarray(arr.reshape(128, NTILES * 576))


# ---------------------------------------------------------------- bass build
_WAIT_CARRIERS = ("InstEventSemaphore", "InstNoOp",
                  "InstUnconditionalBranch", "InstCompareAndBranch")


def _legalize_waits(nc):
    """Split excess semaphore waits onto same-engine NoOp carriers.

    The walrus codegen used by the bass2jax path allows at most 1 attached
    wait on compute instructions and 2 on DMA; Tile sometimes emits more.
    Engines execute instructions in order, so a preceding NoOp carrying the
    extra waits is equivalent.
    """
    uid = 0
    for blk in nc.m.functions[0].blocks:
        insts = blk.instructions
        i = 0
        while i < len(insts):
            inst = insts[i]
            ty = type(inst).__name__
            si = inst.sync_info
            if si is None or ty in _WAIT_CARRIERS:
                i += 1
                continue
            waits = list(si.on_wait or [])
            limit = 1
            if len(waits) <= limit:
                i += 1
                continue
            extra, keep = waits[:-limit], waits[-limit:]
            for w in extra:
                nop = mybir.InstNoOp(
                    name=f"waitnop-{uid}", engine=inst.engine,
                    sync_info=mybir.SyncInfo(on_wait=[w], on_update=[]))
                uid += 1
                insts.insert(i, nop)
                i += 1
            inst.sync_info = mybir.SyncInfo(
                on_wait=keep, on_update=list(si.on_update or []))
            i += 1


def build_nc(skip_conv=False, skip_low=False, skip_fc1=False, skip_fc23=False):
    nc = bass.Bass()
    xth_d = nc.declare_dram_parameter("xth", [128, NTILES * 576], BF16, isOutput=False)
    xl_d = nc.declare_dram_parameter("xl", [128, 24 * BC], BF16, isOutput=False)
    wc_d = nc.declare_dram_parameter("wc", [128, 2048], BF16, isOutput=False)
    wlow_d = nc.declare_dram_parameter("wlow", [128, 24 * 256], BF16, isOutput=False)
    w1_d = nc.declare_dram_parameter("w1r", [33, 128, 1024], BF16, isOutput=False)
    w2_d = nc.declare_dram_parameter("w2r", [128, 2048], BF16, isOutput=False)
    w3_d = nc.declare_dram_parameter("w3r", [128, 10], BF16, isOutput=False)
    b1_d = nc.declare_dram_parameter("b1r", [128, 8], FP32, isOutput=False)
    b2_d = nc.declare_dram_parameter("b2r", [128, 2], FP32, isOutput=False)
    b3_d = nc.declare_dram_parameter("b3r", [5, 1], FP32, isOutput=False)
    bl_d = nc.declare_dram_parameter("blowr", [64, 1], FP32, isOutput=False)
    y_d = nc.declare_dram_parameter("y", [5, 512], FP32, isOutput=True)

    RELU = mybir.ActivationFunctionType.Relu
    MAX = AluOpType.max

    with tile.TileContext(nc) as tc:
        with (
            tc.tile_pool(name="persist", bufs=1) as pp,
            tc.tile_pool(name="work", bufs=3) as wp,
            tc.tile_pool(name="w1pool", bufs=4) as w1p,
        ):
            # conv weights first: the first matmul depends only on these
            wc_sb = pp.tile([128, 2048], BF16, tag="wc")
            nc.sync.dma_start(out=wc_sb[:], in_=wc_d[:])
            # everything below overlaps with the conv phase
            xl_sb = pp.tile([128, 24 * BC], BF16, tag="xl")
            for q in range(3):
                sl = slice(q * 4096, (q + 1) * 4096)
                nc.sync.dma_start(out=xl_sb[:, sl], in_=xl_d[:, sl])
            wlow_sb = pp.tile([128, 24 * 256], BF16, tag="wlow")
            nc.sync.dma_start(out=wlow_sb[:], in_=wlow_d[:])
            w2_sb = pp.tile([128, 2048], BF16, tag="w2")
            nc.sync.dma_start(out=w2_sb[:], in_=w2_d[:])
            w3_sb = pp.tile([128, 10], BF16, tag="w3")
            nc.sync.dma_start(out=w3_sb[:], in_=w3_d[:])
            b1_sb = pp.tile([128, 8], FP32, tag="b1")
            nc.sync.dma_start(out=b1_sb[:], in_=b1_d[:])
            b2_sb = pp.tile([128, 2], FP32, tag="b2")
            nc.sync.dma_start(out=b2_sb[:], in_=b2_d[:])
            b3_sb = pp.tile([5, 1], FP32, tag="b3")
            nc.sync.dma_start(out=b3_sb[:], in_=b3_d[:])
            bl_sb = pp.tile([64, 1], FP32, tag="bl")
            nc.sync.dma_start(out=bl_sb[:], in_=bl_d[:])

            # pooled high features, j2-major: ph[p, j2*512 + t*16 + b]
            ph0 = pp.tile([128, 8192], BF16, tag="ph0")
            ph1 = pp.tile([128, 8192], BF16, tag="ph1")
            xs4a = pp.tile([128, 800], BF16, tag="xs4a")
            xs4b = pp.tile([128, 800], BF16, tag="xs4b")
            nc.gpsimd.memset(xs4a[:], 0.0)
            nc.gpsimd.memset(xs4b[:], 0.0)
            xlow_sb = pp.tile([128, 512], BF16, tag="xlow")
            nc.gpsimd.memset(xlow_sb[:], 0.0)
            h1_sb = pp.tile([128, 8 * 512], BF16, tag="h1")
            h2_sb = pp.tile([128, 2 * 512], BF16, tag="h2")
            y_sb = pp.tile([5, 512], FP32, tag="ysb")

            ph0v = ph0[:].rearrange("p (j t b) -> p j t b", j=16, t=NTILES, b=TIMGS)
            ph1v = ph1[:].rearrange("p (j t b) -> p j t b", j=16, t=NTILES, b=TIMGS)

            # ---------------- conv + pool over 32 tiles
            with tc.tile_pool(name="cpsum", bufs=8, space="PSUM") as cps:
                for t in range(0 if skip_conv else NTILES):
                    xs4 = xs4a if t % 2 == 0 else xs4b
                    # DMA the channel-summed skew tile straight into the
                    # 48-stride layout (36 valid cols per 48-col block)
                    dst = (
                        xs4[:, 8:8 + 768]
                        .rearrange("p (b j) -> p b j", j=48)[:, :, 0:36]
                    )
                    nc.gpsimd.dma_start(
                        out=dst,
                        in_=xth_d[:, t * 576:(t + 1) * 576]
                        .rearrange("p (b j) -> p b j", j=36))

                    cp = [cps.tile([128, 512], FP32, tag="cp", name=f"cp{t}_{i}") for i in range(4)]
                    for kc in range(4):
                        off = 4 * kc + 4
                        rhs = (
                            xs4[:, off:off + 768]
                            .rearrange("p (b j) -> p b j", j=48)[:, :, 0:32]
                        )
                        for mc in range(4):
                            nc.tensor.matmul(
                                cp[mc][:],
                                wc_sb[:, kc * 512 + mc * 128: kc * 512 + (mc + 1) * 128],
                                rhs,
                                start=(kc == 0),
                                stop=(kc == 3),
                            )
                    # relu on scalar (PSUM->SBUF, 1 read each), pool on vector
                    s = [wp.tile([128, 512], BF16, tag=f"s{i}", name=f"s{t}_{i}")
                         for i in range(4)]
                    for i in range(4):
                        nc.scalar.activation(out=s[i][:], in_=cp[i][:], func=RELU)
                    m0 = wp.tile([128, 512], BF16, tag="m0")
                    m1 = wp.tile([128, 512], BF16, tag="m1")
                    nc.vector.tensor_max(out=m0[:], in0=s[0][:], in1=s[2][:])
                    nc.vector.tensor_max(out=m1[:], in0=s[1][:], in1=s[3][:])
                    m0v = m0[:].rearrange("p (b j t) -> p j b t", j=16, t=2)
                    m1v = m1[:].rearrange("p (b j t) -> p j b t", j=16, t=2)
                    nc.vector.tensor_max(
                        out=ph0v[:, :, t, :], in0=m0v[:, :, :, 0], in1=m0v[:, :, :, 1])
                    nc.vector.tensor_max(
                        out=ph1v[:, :, t, :], in0=m1v[:, :, :, 0], in1=m1v[:, :, :, 1])

            # ---------------- low branch
            with tc.tile_pool(name="lpsum", bufs=1, space="PSUM") as lps:
                lpA = lps.tile([128, 512], FP32, tag="lpA")
                lpB = lps.tile([128, 512], FP32, tag="lpB")
                for kc in range(0 if skip_low else 24):
                    rhs = xl_sb[:, kc * 512:(kc + 1) * 512]
                    nc.tensor.matmul(
                        lpA[:], wlow_sb[:, kc * 256:kc * 256 + 128], rhs,
                        start=(kc == 0), stop=(kc == 23))
                    nc.tensor.matmul(
                        lpB[:], wlow_sb[:, kc * 256 + 128:(kc + 1) * 256], rhs,
                        start=(kc == 0), stop=(kc == 23))
                sB = wp.tile([128, 512], BF16, tag="sB")
                nc.scalar.activation(
                    out=sB[:], in_=lpB[:],
                    func=mybir.ActivationFunctionType.Copy)
                mAB = wp.tile([128, 512], BF16, tag="mAB")
                nc.vector.tensor_max(out=mAB[:], in0=lpA[:], in1=sB[:])
                # partition shift via sbuf->sbuf DMA, then final pool max
                tmp = wp.tile([64, 512], BF16, tag="ltmp")
                nc.sync.dma_start(out=tmp[:], in_=mAB[64:128, :])
                mm64 = wp.tile([64, 512], BF16, tag="mm64")
                nc.vector.tensor_max(out=mm64[:], in0=mAB[0:64, :], in1=tmp[:])
                nc.scalar.activation(
                    out=xlow_sb[0:64, :], in_=mm64[:], func=RELU,
                    bias=bl_sb[:, 0:1])

            # ---------------- FC1 (weights streamed)
            with tc.tile_pool(name="fpsum", bufs=1, space="PSUM") as fps:
                fp = [fps.tile([128, 512], FP32, tag=f"fp{i}", name=f"fp{i}") for i in range(8)]
                for ks in range(0 if skip_fc1 else 33):
                    w1t = w1p.tile([128, 1024], BF16, tag="w1t", bufs=8)
                    nc.gpsimd.dma_start(out=w1t[:], in_=w1_d[ks])
                    if ks < 32:
                        j2, c = ks // 2, ks % 2
                        src = ph0 if c == 0 else ph1
                        rhs = src[:, j2 * 512:(j2 + 1) * 512]
                    else:
                        rhs = xlow_sb[:]
                    for mc in range(8):
                        nc.tensor.matmul(
                            fp[mc][:], w1t[:, mc * 128:(mc + 1) * 128], rhs,
                            start=(ks == 0), stop=(ks == 32))
                for mc in range(8):
                    nc.scalar.activation(
                        out=h1_sb[:, mc * 512:(mc + 1) * 512], in_=fp[mc][:],
                        func=RELU, bias=b1_sb[:, mc:mc + 1])

            # ---------------- FC2 + FC3
            with tc.tile_pool(name="gpsum", bufs=1, space="PSUM") as gps:
                gp = [gps.tile([128, 512], FP32, tag=f"gp{i}", name=f"gp{i}") for i in range(2)]
                for kc in range(0 if skip_fc23 else 8):
                    rhs = h1_sb[:, kc * 512:(kc + 1) * 512]
                    for mc in range(2):
                        nc.tensor.matmul(
                            gp[mc][:],
                            w2_sb[:, kc * 256 + mc * 128: kc * 256 + (mc + 1) * 128],
                            rhs, start=(kc == 0), stop=(kc == 7))
                for mc in range(2):
                    nc.scalar.activation(
                        out=h2_sb[:, mc * 512:(mc + 1) * 512], in_=gp[mc][:],
                        func=RELU, bias=b2_sb[:, mc:mc + 1])

                yp = gps.tile([128, 512], FP32, tag="yp")
                for kc in range(2):
                    nc.tensor.matmul(
                        yp[0:5, :], w3_sb[:, kc * 5:(kc + 1) * 5],
                        h2_sb[:, kc * 512:(kc + 1) * 512],
                        start=(kc == 0), stop=(kc == 1))
                nc.vector.tensor_scalar_add(
                    out=y_sb[:], in0=yp[0:5, :], scalar1=b3_sb[:, 0:1])
                nc.sync.dma_start(out=y_d[:], in_=y_sb[:])

    _legalize_waits(nc)
    return nc


_NC = None
TRACE = False
TRACE_DIR = None
LAST_RESULT = None


def kernel(**inputs):
    global _NC, LAST_RESULT
    w = build_weights(inputs)
    if _NC is None:
        _NC = build_nc()
    x = np.asarray(inputs["x"], np.float32)
    in_maps = []
    for c in range(NCORES):
        xs_c = x[c * BC:(c + 1) * BC]
        m = {"xl": pack_x_low(xs_c), "xth": pack_x_high(xs_c)}
        m.update(w)
        in_maps.append(m)
    res = run_bass_kernel_spmd(_NC, in_maps, list(range(NCORES)), trace=TRACE,
                               tmpdir=TRACE_DIR)
    LAST_RESULT = res
    y = np.concatenate(
        [np.asarray(res.results[i]["y"], np.float32).T for i in range(NCORES)], axis=0)
    return y


# revision 25
# speedup vs baseline: 1.1597x; 1.1597x over previous
"""Trainium2 Bass kernel for nn_MixedResolutionCNN.

Network (per sample, eval mode):
  high branch: ridgelet conv 3->16 k=15 same-pad (kernel broadcast over in-ch)
               -> relu -> maxpool2 -> 4096 feats
  low branch:  bilinear resize 32->8 -> conv 3->4 k=3 pad1 + bias -> relu
               -> maxpool2 -> 64 feats
  head:        concat -> fc 4160->1024 relu -> 1024->256 relu -> 256->5

Device strategy (pure data parallel over 8 cores, 512 images/core):

* The ridgelet kernel is identical across the 3 input channels, so the high
  conv contracts the channel-summed image xs = sum_c x[:,c] with a 16x15x15
  kernel. Expressed as matmuls with contraction over (v, i') = (kernel col,
  image row): out[(o,i),(b,j)] = sum Khat[o, i'-i+7, v] * xs[b, i', j+v-7].
  The moving operand for v-chunk kc is a skewed 4x replication of the
  column-padded image rows: block dv holds xs shifted by dv columns so a
  single strided AP reads xs[b, i', j + (4kc+dv) - 7] for all 128
  partitions.  4 K-chunks x 4 M-chunks of [128,128,512] matmuls per
  16-image tile.
* relu/maxpool fold into the pool maxes (relu(max(a,b)) == max(0,a,b) via
  one scalar_tensor_tensor op on the vector engine); pooled features are
  written j2-major so every FC1 rhs chunk is a fully contiguous [128,512]
  slab (peak-rate matmul feed).  Channel-sum adds run on gpsimd to keep the
  vector engine under the tensor-engine roofline.
* low branch: resize+conv fold into one linear map [3072, 256]; x is packed
  host-side as [feature, batch] so the 24 K-chunks stream contiguously.
  Output partitions hold the 4 pool-parity groups (2 matmuls of 128).
* FC1 weights (8.5MB bf16) stream from DRAM, double buffered; everything
  else is resident in SBUF.  Conv weights are DMA'd first so the tensor
  engine starts within ~2us.
"""

import numpy as np
import ml_dtypes

import concourse.bass as bass
import concourse.tile as tile
from concourse import mybir
from concourse.alu_op_type import AluOpType
from concourse.bass_utils import run_bass_kernel_spmd

BF16NP = ml_dtypes.bfloat16
FP32 = mybir.dt.float32
BF16 = mybir.dt.bfloat16

B = 4096
NCORES = 8
BC = B // NCORES           # 512 images per core
TIMGS = 16                 # images per tile
NTILES = BC // TIMGS       # 32
KS = 15
OUT_CH = 16


# ---------------------------------------------------------------- host math
def _ridgelet_kernel(r_dirs, r_scales, r_pos):
    """[16,15,15] channel-shared ridgelet kernel, mirrors reference."""
    c = np.arange(KS, dtype=np.float32) - KS // 2
    x1 = c[:, None]
    x2 = c[None, :]
    d = np.asarray(r_dirs, np.float32)[:, None, None]
    s = np.asarray(r_scales, np.float32)[:, None, None]
    p = np.asarray(r_pos, np.float32)[:, None, None]
    t = (x1 * np.cos(d) + x2 * np.sin(d) - p) / s
    vals = np.exp(-t * t / 2.0) - 0.5 * np.exp(-t * t / 8.0)
    return vals.reshape(OUT_CH, 10, KS, KS).sum(axis=1)


def _resize_mat(in_size=32, out_size=8):
    """Row matrix of jax.image.resize(..., 'bilinear', antialias=True)."""
    scale = out_size / in_size
    inv = 1.0 / scale
    kscale = max(inv, 1.0)
    sample_f = (np.arange(out_size, dtype=np.float64) + 0.5) * inv - 0.5
    x = np.abs(sample_f[None, :] - np.arange(in_size, dtype=np.float64)[:, None])
    w = np.maximum(0.0, 1.0 - x / kscale)
    w = w / w.sum(axis=0, keepdims=True)
    return w.T.astype(np.float32)  # [out, in]


def build_weights(inputs):
    """All packed device arrays (shared across cores)."""
    khat = _ridgelet_kernel(inputs["r_dirs"], inputs["r_scales"], inputs["r_pos"])
    # padded to 16x16 so v=15 / u out-of-range index to a zero slot
    khat_p = np.zeros((OUT_CH, 16, 16), np.float32)
    khat_p[:, :KS, :KS] = khat

    # conv lhsT: wc[p=(dv,i'), kc*512 + ch*128 + wi]
    dvip = np.arange(128)
    dv = dvip // 32
    ip = dvip % 32
    m = np.arange(512)
    ch = m // 128
    wi = m % 128
    par = ch // 2          # i parity (0=even rows, 1=odd)
    oh = ch % 2            # o half
    o = oh * 8 + wi // 16
    i2 = wi % 16
    i = 2 * i2 + par
    wc = np.zeros((128, 2048), np.float32)
    u = ip[:, None] - i[None, :] + 7          # [128, 512]
    umask = (u >= 0) & (u < KS)
    uc = np.clip(u, 0, 15)
    for kc in range(4):
        v = 4 * kc + dv                        # [128]
        vals = khat_p[o[None, :], uc, np.clip(v, 0, 15)[:, None]]
        vals = np.where(umask, vals, 0.0)
        wc[:, kc * 512:(kc + 1) * 512] = vals

    # low branch: fold resize+conv into [3072, 256]
    A = _resize_mat()
    Ash = np.zeros((3, 8, 32), np.float32)
    for dh in range(3):
        for ph in range(8):
            r = ph + dh - 1
            if 0 <= r < 8:
                Ash[dh, ph] = A[r]
    wlow = np.asarray(inputs["wlow"], np.float32)
    # D[c,i,w,o,ph,pw] = sum_{dh,dw} wlow[o,c,dh,dw] Ash[dh,ph,i] Ash[dw,pw,w]
    D = np.einsum("ocuv,upi,vqw->ciwopq", wlow, Ash, Ash).astype(np.float32)
    Dp = D.reshape(3072, 4, 8, 8)              # [(c,i,w), o, ph, pw]
    # out col layout: 2 matmuls of 128.  Pool partners sit at the SAME
    # partition in the two PSUM banks (A holds groups 0,2; B holds 1,3) so
    # the first pool max never crosses partitions.
    Wn = np.zeros((3072, 2, 128), np.float32)
    G = [(0, 0), (0, 1), (1, 0), (1, 1)]
    for g, (pp_, qq) in enumerate(G):
        blk = Dp[:, :, pp_::2, qq::2].reshape(3072, 64)
        Wn[:, g % 2, (g // 2) * 64:(g // 2) * 64 + 64] = blk
    wlowp = np.ascontiguousarray(
        Wn.reshape(24, 128, 256).transpose(1, 0, 2).reshape(128, 24 * 256))

    # FC1 reorder: kstep = j2*2 + chunk over high feats, kstep 32 = low
    w1 = np.asarray(inputs["w1"], np.float32)          # [1024, 4160]
    w1hi = w1[:, 64:].reshape(1024, 16, 16, 16)        # [n, o, i2, j2]
    w1r = np.zeros((33, 128, 1024), np.float32)
    for ks in range(32):
        j2, c = ks // 2, ks % 2
        blk = w1hi[:, 8 * c:8 * (c + 1), :, j2]        # [n, 8, 16]
        w1r[ks] = blk.reshape(1024, 128).T
    w1r[32, :64, :] = w1[:, :64].T

    w2 = np.asarray(inputs["w2"], np.float32)          # [256, 1024]
    w2r = np.zeros((128, 2048), np.float32)
    for kc in range(8):
        w2r[:, kc * 256:(kc + 1) * 256] = w2[:, kc * 128:(kc + 1) * 128].T
    w3 = np.asarray(inputs["w3"], np.float32)          # [5, 256]
    w3r = np.zeros((128, 10), np.float32)
    for kc in range(2):
        w3r[:, kc * 5:(kc + 1) * 5] = w3[:, kc * 128:(kc + 1) * 128].T

    b1r = np.asarray(inputs["b1"], np.float32).reshape(8, 128).T.copy()
    b2r = np.asarray(inputs["b2"], np.float32).reshape(2, 128).T.copy()
    b3r = np.asarray(inputs["b3"], np.float32)[:, None].copy()
    blowr = np.repeat(np.asarray(inputs["blow"], np.float32), 16)[:, None].copy()

    return {
        "wc": wc.astype(BF16NP),
        "wlow": wlowp.astype(BF16NP),
        "w1r": w1r.astype(BF16NP),
        "w2r": w2r.astype(BF16NP),
        "w3r": w3r.astype(BF16NP),
        "b1r": np.ascontiguousarray(b1r),
        "b2r": np.ascontiguousarray(b2r),
        "b3r": b3r,
        "blowr": np.ascontiguousarray(blowr),
    }


def pack_x_low(x_core):
    """[512,3,32,32] f32 -> [128, 24*512] bf16: xl[p, kc*512+tb] =
    x[tb, (kc*128+p)//1024, ...] i.e. feature-major transpose."""
    xc = np.asarray(x_core, np.float32).astype(BF16NP)
    arr = xc.transpose(1, 2, 3, 0).reshape(3072, BC)      # [(c,i,w), tb]
    return np.ascontiguousarray(
        arr.reshape(24, 128, BC).transpose(1, 0, 2).reshape(128, 24 * BC))


def pack_x_high(x_core):
    """4x skew-replicated, channel-summed high-branch input with the zero
    margins baked in: per tile a contiguous [128, 800] slab laid out as
    xh4[dv*32+i, t*800 + 8 + 48*b + jj] = sum_c x[t*16+b, c, i, jj+dv-3]
    for jj in [0,36), zeros elsewhere.  DMA'd contiguously on device.
    """
    xs = np.asarray(x_core, np.float32).sum(axis=1).astype(BF16NP)  # [BC,32,32]
    xpad = np.zeros((BC, 32, 42), BF16NP)
    xpad[:, :, 3:35] = xs
    arr = np.stack([xpad[:, :, dv:dv + 36] for dv in range(4)])
    arr = arr.reshape(4, NTILES, TIMGS, 32, 36).transpose(0, 3, 1, 2, 4)
    blocks = np.zeros((128, NTILES, TIMGS, 48), BF16NP)
    blocks[:, :, :, 0:36] = arr.reshape(128, NTILES, TIMGS, 36)
    full = np.zeros((128, NTILES, 800), BF16NP)
    full[:, :, 8:8 + 768] = blocks.reshape(128, NTILES, 768)
    return np.ascontiguousarray(full.reshape(128, NTILES * 800))


# ---------------------------------------------------------------- bass build
_WAIT_CARRIERS = ("InstEventSemaphore", "InstNoOp",
                  "InstUnconditionalBranch", "InstCompareAndBranch")


def _legalize_waits(nc):
    """Split excess semaphore waits onto same-engine NoOp carriers.

    The walrus codegen used by the bass2jax path allows at most 1 attached
    wait on compute instructions and 2 on DMA; Tile sometimes emits more.
    Engines execute instructions in order, so a preceding NoOp carrying the
    extra waits is equivalent.
    """
    uid = 0
    for blk in nc.m.functions[0].blocks:
        insts = blk.instructions
        i = 0
        while i < len(insts):
            inst = insts[i]
            ty = type(inst).__name__
            si = inst.sync_info
            if si is None or ty in _WAIT_CARRIERS:
                i += 1
                continue
            waits = list(si.on_wait or [])
            limit = 1
            if len(waits) <= limit:
                i += 1
                continue
            extra, keep = waits[:-limit], waits[-limit:]
            for w in extra:
                nop = mybir.InstNoOp(
                    name=f"waitnop-{uid}", engine=inst.engine,
                    sync_info=mybir.SyncInfo(on_wait=[w], on_update=[]))
                uid += 1
                insts.insert(i, nop)
                i += 1
            inst.sync_info = mybir.SyncInfo(
                on_wait=keep, on_update=list(si.on_update or []))
            i += 1


def build_nc(skip_conv=False, skip_low=False, skip_fc1=False, skip_fc23=False):
    nc = bass.Bass()
    xth_d = nc.declare_dram_parameter("xth", [128, NTILES * 800], BF16, isOutput=False)
    xl_d = nc.declare_dram_parameter("xl", [128, 24 * BC], BF16, isOutput=False)
    wc_d = nc.declare_dram_parameter("wc", [128, 2048], BF16, isOutput=False)
    wlow_d = nc.declare_dram_parameter("wlow", [128, 24 * 256], BF16, isOutput=False)
    w1_d = nc.declare_dram_parameter("w1r", [33, 128, 1024], BF16, isOutput=False)
    w2_d = nc.declare_dram_parameter("w2r", [128, 2048], BF16, isOutput=False)
    w3_d = nc.declare_dram_parameter("w3r", [128, 10], BF16, isOutput=False)
    b1_d = nc.declare_dram_parameter("b1r", [128, 8], FP32, isOutput=False)
    b2_d = nc.declare_dram_parameter("b2r", [128, 2], FP32, isOutput=False)
    b3_d = nc.declare_dram_parameter("b3r", [5, 1], FP32, isOutput=False)
    bl_d = nc.declare_dram_parameter("blowr", [64, 1], FP32, isOutput=False)
    y_d = nc.declare_dram_parameter("y", [5, 512], FP32, isOutput=True)

    RELU = mybir.ActivationFunctionType.Relu
    MAX = AluOpType.max

    with tile.TileContext(nc) as tc:
        with (
            tc.tile_pool(name="persist", bufs=1) as pp,
            tc.tile_pool(name="work", bufs=3) as wp,
            tc.tile_pool(name="w1pool", bufs=4) as w1p,
        ):
            # conv weights first: the first matmul depends only on these
            wc_sb = pp.tile([128, 2048], BF16, tag="wc")
            nc.sync.dma_start(out=wc_sb[:], in_=wc_d[:])
            # everything below overlaps with the conv phase
            xl_sb = pp.tile([128, 24 * BC], BF16, tag="xl")
            for q in range(3):
                sl = slice(q * 4096, (q + 1) * 4096)
                nc.sync.dma_start(out=xl_sb[:, sl], in_=xl_d[:, sl])
            wlow_sb = pp.tile([128, 24 * 256], BF16, tag="wlow")
            nc.sync.dma_start(out=wlow_sb[:], in_=wlow_d[:])
            w2_sb = pp.tile([128, 2048], BF16, tag="w2")
            nc.sync.dma_start(out=w2_sb[:], in_=w2_d[:])
            w3_sb = pp.tile([128, 10], BF16, tag="w3")
            nc.sync.dma_start(out=w3_sb[:], in_=w3_d[:])
            b1_sb = pp.tile([128, 8], FP32, tag="b1")
            nc.sync.dma_start(out=b1_sb[:], in_=b1_d[:])
            b2_sb = pp.tile([128, 2], FP32, tag="b2")
            nc.sync.dma_start(out=b2_sb[:], in_=b2_d[:])
            b3_sb = pp.tile([5, 1], FP32, tag="b3")
            nc.sync.dma_start(out=b3_sb[:], in_=b3_d[:])
            bl_sb = pp.tile([64, 1], FP32, tag="bl")
            nc.sync.dma_start(out=bl_sb[:], in_=bl_d[:])

            # pooled high features, j2-major: ph[p, j2*512 + t*16 + b]
            ph0 = pp.tile([128, 8192], BF16, tag="ph0")
            ph1 = pp.tile([128, 8192], BF16, tag="ph1")
            xlow_sb = pp.tile([128, 512], BF16, tag="xlow")
            nc.gpsimd.memset(xlow_sb[:], 0.0)
            h1_sb = pp.tile([128, 8 * 512], BF16, tag="h1")
            h2_sb = pp.tile([128, 2 * 512], BF16, tag="h2")
            y_sb = pp.tile([5, 512], FP32, tag="ysb")

            ph0v = ph0[:].rearrange("p (j t b) -> p j t b", j=16, t=NTILES, b=TIMGS)
            ph1v = ph1[:].rearrange("p (j t b) -> p j t b", j=16, t=NTILES, b=TIMGS)

            # ---------------- conv + pool over 32 tiles
            with tc.tile_pool(name="cpsum", bufs=8, space="PSUM") as cps:
                for t in range(0 if skip_conv else NTILES):
                    # contiguous DMA of the pre-padded skew tile
                    xs4 = wp.tile([128, 800], BF16, tag="xs4", bufs=4)
                    nc.gpsimd.dma_start(
                        out=xs4[:], in_=xth_d[:, t * 800:(t + 1) * 800])

                    cp = [cps.tile([128, 512], FP32, tag="cp", name=f"cp{t}_{i}") for i in range(4)]
                    for kc in range(4):
                        off = 4 * kc + 4
                        rhs = (
                            xs4[:, off:off + 768]
                            .rearrange("p (b j) -> p b j", j=48)[:, :, 0:32]
                        )
                        for mc in range(4):
                            nc.tensor.matmul(
                                cp[mc][:],
                                wc_sb[:, kc * 512 + mc * 128: kc * 512 + (mc + 1) * 128],
                                rhs,
                                start=(kc == 0),
                                stop=(kc == 3),
                            )
                    # relu on scalar (PSUM->SBUF, 1 read each), pool on vector
                    s = [wp.tile([128, 512], BF16, tag=f"s{i}", name=f"s{t}_{i}")
                         for i in range(4)]
                    for i in range(4):
                        nc.scalar.activation(out=s[i][:], in_=cp[i][:], func=RELU)
                    m0 = wp.tile([128, 512], BF16, tag="m0")
                    m1 = wp.tile([128, 512], BF16, tag="m1")
                    nc.vector.tensor_max(out=m0[:], in0=s[0][:], in1=s[2][:])
                    nc.vector.tensor_max(out=m1[:], in0=s[1][:], in1=s[3][:])
                    m0v = m0[:].rearrange("p (b j t) -> p j b t", j=16, t=2)
                    m1v = m1[:].rearrange("p (b j t) -> p j b t", j=16, t=2)
                    nc.vector.tensor_max(
                        out=ph0v[:, :, t, :], in0=m0v[:, :, :, 0], in1=m0v[:, :, :, 1])
                    nc.vector.tensor_max(
                        out=ph1v[:, :, t, :], in0=m1v[:, :, :, 0], in1=m1v[:, :, :, 1])

            # ---------------- low branch
            with tc.tile_pool(name="lpsum", bufs=1, space="PSUM") as lps:
                lpA = lps.tile([128, 512], FP32, tag="lpA")
                lpB = lps.tile([128, 512], FP32, tag="lpB")
                for kc in range(0 if skip_low else 24):
                    rhs = xl_sb[:, kc * 512:(kc + 1) * 512]
                    nc.tensor.matmul(
                        lpA[:], wlow_sb[:, kc * 256:kc * 256 + 128], rhs,
                        start=(kc == 0), stop=(kc == 23))
                    nc.tensor.matmul(
                        lpB[:], wlow_sb[:, kc * 256 + 128:(kc + 1) * 256], rhs,
                        start=(kc == 0), stop=(kc == 23))
                sB = wp.tile([128, 512], BF16, tag="sB")
                nc.scalar.activation(
                    out=sB[:], in_=lpB[:],
                    func=mybir.ActivationFunctionType.Copy)
                mAB = wp.tile([128, 512], BF16, tag="mAB")
                nc.vector.tensor_max(out=mAB[:], in0=lpA[:], in1=sB[:])
                # partition shift via sbuf->sbuf DMA, then final pool max
                tmp = wp.tile([64, 512], BF16, tag="ltmp")
                nc.sync.dma_start(out=tmp[:], in_=mAB[64:128, :])
                mm64 = wp.tile([64, 512], BF16, tag="mm64")
                nc.vector.tensor_max(out=mm64[:], in0=mAB[0:64, :], in1=tmp[:])
                nc.scalar.activation(
                    out=xlow_sb[0:64, :], in_=mm64[:], func=RELU,
                    bias=bl_sb[:, 0:1])

            # ---------------- FC1 (weights streamed)
            with tc.tile_pool(name="fpsum", bufs=1, space="PSUM") as fps:
                fp = [fps.tile([128, 512], FP32, tag=f"fp{i}", name=f"fp{i}") for i in range(8)]
                for ks in range(0 if skip_fc1 else 33):
                    w1t = w1p.tile([128, 1024], BF16, tag="w1t", bufs=8)
                    nc.gpsimd.dma_start(out=w1t[:], in_=w1_d[ks])
                    if ks < 32:
                        j2, c = ks // 2, ks % 2
                        src = ph0 if c == 0 else ph1
                        rhs = src[:, j2 * 512:(j2 + 1) * 512]
                    else:
                        rhs = xlow_sb[:]
                    for mc in range(8):
                        nc.tensor.matmul(
                            fp[mc][:], w1t[:, mc * 128:(mc + 1) * 128], rhs,
                            start=(ks == 0), stop=(ks == 32))
                for mc in range(8):
                    nc.scalar.activation(
                        out=h1_sb[:, mc * 512:(mc + 1) * 512], in_=fp[mc][:],
                        func=RELU, bias=b1_sb[:, mc:mc + 1])

            # ---------------- FC2 + FC3
            with tc.tile_pool(name="gpsum", bufs=1, space="PSUM") as gps:
                gp = [gps.tile([128, 512], FP32, tag=f"gp{i}", name=f"gp{i}") for i in range(2)]
                for kc in range(0 if skip_fc23 else 8):
                    rhs = h1_sb[:, kc * 512:(kc + 1) * 512]
                    for mc in range(2):
                        nc.tensor.matmul(
                            gp[mc][:],
                            w2_sb[:, kc * 256 + mc * 128: kc * 256 + (mc + 1) * 128],
                            rhs, start=(kc == 0), stop=(kc == 7))
                for mc in range(2):
                    nc.scalar.activation(
                        out=h2_sb[:, mc * 512:(mc + 1) * 512], in_=gp[mc][:],
                        func=RELU, bias=b2_sb[:, mc:mc + 1])

                yp = gps.tile([128, 512], FP32, tag="yp")
                for kc in range(2):
                    nc.tensor.matmul(
                        yp[0:5, :], w3_sb[:, kc * 5:(kc + 1) * 5],
                        h2_sb[:, kc * 512:(kc + 1) * 512],
                        start=(kc == 0), stop=(kc == 1))
                nc.vector.tensor_scalar_add(
                    out=y_sb[:], in0=yp[0:5, :], scalar1=b3_sb[:, 0:1])
                nc.sync.dma_start(out=y_d[:], in_=y_sb[:])

    _legalize_waits(nc)
    return nc


_NC = None
TRACE = False
TRACE_DIR = None
LAST_RESULT = None


def kernel(**inputs):
    global _NC, LAST_RESULT
    w = build_weights(inputs)
    if _NC is None:
        _NC = build_nc()
    x = np.asarray(inputs["x"], np.float32)
    in_maps = []
    for c in range(NCORES):
        xs_c = x[c * BC:(c + 1) * BC]
        m = {"xl": pack_x_low(xs_c), "xth": pack_x_high(xs_c)}
        m.update(w)
        in_maps.append(m)
    res = run_bass_kernel_spmd(_NC, in_maps, list(range(NCORES)), trace=TRACE,
                               tmpdir=TRACE_DIR)
    LAST_RESULT = res
    y = np.concatenate(
        [np.asarray(res.results[i]["y"], np.float32).T for i in range(NCORES)], axis=0)
    return y


# revision 32
# speedup vs baseline: 1.1771x; 1.0150x over previous
"""Trainium2 Bass kernel for nn_MixedResolutionCNN.

Network (per sample, eval mode):
  high branch: ridgelet conv 3->16 k=15 same-pad (kernel broadcast over in-ch)
               -> relu -> maxpool2 -> 4096 feats
  low branch:  bilinear resize 32->8 -> conv 3->4 k=3 pad1 + bias -> relu
               -> maxpool2 -> 64 feats
  head:        concat -> fc 4160->1024 relu -> 1024->256 relu -> 256->5

Device strategy (pure data parallel over 8 cores, 512 images/core):

* The ridgelet kernel is identical across the 3 input channels, so the high
  conv contracts the channel-summed image xs = sum_c x[:,c] with a 16x15x15
  kernel. Expressed as matmuls with contraction over (v, i') = (kernel col,
  image row): out[(o,i),(b,j)] = sum Khat[o, i'-i+7, v] * xs[b, i', j+v-7].
  The moving operand for v-chunk kc is a skewed 4x replication of the
  column-padded image rows: block dv holds xs shifted by dv columns so a
  single strided AP reads xs[b, i', j + (4kc+dv) - 7] for all 128
  partitions.  4 K-chunks x 4 M-chunks of [128,128,512] matmuls per
  16-image tile.
* relu/maxpool fold into the pool maxes (relu(max(a,b)) == max(0,a,b) via
  one scalar_tensor_tensor op on the vector engine); pooled features are
  written j2-major so every FC1 rhs chunk is a fully contiguous [128,512]
  slab (peak-rate matmul feed).  Channel-sum adds run on gpsimd to keep the
  vector engine under the tensor-engine roofline.
* low branch: resize+conv fold into one linear map [3072, 256]; x is packed
  host-side as [feature, batch] so the 24 K-chunks stream contiguously.
  Output partitions hold the 4 pool-parity groups (2 matmuls of 128).
* FC1 weights (8.5MB bf16) stream from DRAM, double buffered; everything
  else is resident in SBUF.  Conv weights are DMA'd first so the tensor
  engine starts within ~2us.
"""

import numpy as np
import ml_dtypes

import concourse.bass as bass
import concourse.tile as tile
from concourse import mybir
from concourse.alu_op_type import AluOpType
from concourse.bass_utils import run_bass_kernel_spmd

BF16NP = ml_dtypes.bfloat16
FP32 = mybir.dt.float32
BF16 = mybir.dt.bfloat16

B = 4096
NCORES = 8
BC = B // NCORES           # 512 images per core
TIMGS = 16                 # images per tile
NTILES = BC // TIMGS       # 32
KS = 15
OUT_CH = 16


# ---------------------------------------------------------------- host math
def _ridgelet_kernel(r_dirs, r_scales, r_pos):
    """[16,15,15] channel-shared ridgelet kernel, mirrors reference."""
    c = np.arange(KS, dtype=np.float32) - KS // 2
    x1 = c[:, None]
    x2 = c[None, :]
    d = np.asarray(r_dirs, np.float32)[:, None, None]
    s = np.asarray(r_scales, np.float32)[:, None, None]
    p = np.asarray(r_pos, np.float32)[:, None, None]
    t = (x1 * np.cos(d) + x2 * np.sin(d) - p) / s
    vals = np.exp(-t * t / 2.0) - 0.5 * np.exp(-t * t / 8.0)
    return vals.reshape(OUT_CH, 10, KS, KS).sum(axis=1)


def _resize_mat(in_size=32, out_size=8):
    """Row matrix of jax.image.resize(..., 'bilinear', antialias=True)."""
    scale = out_size / in_size
    inv = 1.0 / scale
    kscale = max(inv, 1.0)
    sample_f = (np.arange(out_size, dtype=np.float64) + 0.5) * inv - 0.5
    x = np.abs(sample_f[None, :] - np.arange(in_size, dtype=np.float64)[:, None])
    w = np.maximum(0.0, 1.0 - x / kscale)
    w = w / w.sum(axis=0, keepdims=True)
    return w.T.astype(np.float32)  # [out, in]


def build_weights(inputs):
    """All packed device arrays (shared across cores)."""
    khat = _ridgelet_kernel(inputs["r_dirs"], inputs["r_scales"], inputs["r_pos"])
    # padded to 16x16 so v=15 / u out-of-range index to a zero slot
    khat_p = np.zeros((OUT_CH, 16, 16), np.float32)
    khat_p[:, :KS, :KS] = khat

    # conv lhsT: wc[p=(dv,i'), kc*512 + ch*128 + wi]
    dvip = np.arange(128)
    dv = dvip // 32
    ip = dvip % 32
    m = np.arange(512)
    ch = m // 128
    wi = m % 128
    par = ch // 2          # i parity (0=even rows, 1=odd)
    oh = ch % 2            # o half
    o = oh * 8 + wi // 16
    i2 = wi % 16
    i = 2 * i2 + par
    wc = np.zeros((128, 2048), np.float32)
    u = ip[:, None] - i[None, :] + 7          # [128, 512]
    umask = (u >= 0) & (u < KS)
    uc = np.clip(u, 0, 15)
    for kc in range(4):
        v = 4 * kc + dv                        # [128]
        vals = khat_p[o[None, :], uc, np.clip(v, 0, 15)[:, None]]
        vals = np.where(umask, vals, 0.0)
        wc[:, kc * 512:(kc + 1) * 512] = vals

    # low branch: fold resize+conv into [3072, 256]
    A = _resize_mat()
    Ash = np.zeros((3, 8, 32), np.float32)
    for dh in range(3):
        for ph in range(8):
            r = ph + dh - 1
            if 0 <= r < 8:
                Ash[dh, ph] = A[r]
    wlow = np.asarray(inputs["wlow"], np.float32)
    # D[c,i,w,o,ph,pw] = sum_{dh,dw} wlow[o,c,dh,dw] Ash[dh,ph,i] Ash[dw,pw,w]
    D = np.einsum("ocuv,upi,vqw->ciwopq", wlow, Ash, Ash).astype(np.float32)
    Dp = D.reshape(3072, 4, 8, 8)              # [(c,i,w), o, ph, pw]
    # out col layout: 2 matmuls of 128.  Pool partners sit at the SAME
    # partition in the two PSUM banks (A holds groups 0,2; B holds 1,3) so
    # the first pool max never crosses partitions.
    Wn = np.zeros((3072, 2, 128), np.float32)
    G = [(0, 0), (0, 1), (1, 0), (1, 1)]
    for g, (pp_, qq) in enumerate(G):
        blk = Dp[:, :, pp_::2, qq::2].reshape(3072, 64)
        Wn[:, g % 2, (g // 2) * 64:(g // 2) * 64 + 64] = blk
    wlowp = np.ascontiguousarray(
        Wn.reshape(24, 128, 256).transpose(1, 0, 2).reshape(128, 24 * 256))

    # FC1 reorder: kstep = j2*2 + chunk over high feats, kstep 32 = low
    w1 = np.asarray(inputs["w1"], np.float32)          # [1024, 4160]
    w1hi = w1[:, 64:].reshape(1024, 16, 16, 16)        # [n, o, i2, j2]
    w1r = np.zeros((33, 128, 1024), np.float32)
    for ks in range(32):
        j2, c = ks // 2, ks % 2
        blk = w1hi[:, 8 * c:8 * (c + 1), :, j2]        # [n, 8, 16]
        w1r[ks] = blk.reshape(1024, 128).T
    w1r[32, :64, :] = w1[:, :64].T

    w2 = np.asarray(inputs["w2"], np.float32)          # [256, 1024]
    w2r = np.zeros((128, 2048), np.float32)
    for kc in range(8):
        w2r[:, kc * 256:(kc + 1) * 256] = w2[:, kc * 128:(kc + 1) * 128].T
    w3 = np.asarray(inputs["w3"], np.float32)          # [5, 256]
    w3r = np.zeros((128, 10), np.float32)
    for kc in range(2):
        w3r[:, kc * 5:(kc + 1) * 5] = w3[:, kc * 128:(kc + 1) * 128].T

    b1r = np.asarray(inputs["b1"], np.float32).reshape(8, 128).T.copy()
    b2r = np.asarray(inputs["b2"], np.float32).reshape(2, 128).T.copy()
    b3r = np.asarray(inputs["b3"], np.float32)[:, None].copy()
    blowr = np.repeat(np.asarray(inputs["blow"], np.float32), 16)[:, None].copy()

    return {
        "wc": wc.astype(BF16NP),
        "wlow": wlowp.astype(BF16NP),
        "w1r": w1r.astype(BF16NP),
        "w2r": w2r.astype(BF16NP),
        "w3r": w3r.astype(BF16NP),
        "b1r": np.ascontiguousarray(b1r),
        "b2r": np.ascontiguousarray(b2r),
        "b3r": b3r,
        "blowr": np.ascontiguousarray(blowr),
    }


def pack_x_low(x_core):
    """[512,3,32,32] f32 -> [128, 24*512] bf16: xl[p, kc*512+tb] =
    x[tb, (kc*128+p)//1024, ...] i.e. feature-major transpose."""
    xc = np.asarray(x_core, np.float32).astype(BF16NP)
    arr = xc.transpose(1, 2, 3, 0).reshape(3072, BC)      # [(c,i,w), tb]
    return np.ascontiguousarray(
        arr.reshape(24, 128, BC).transpose(1, 0, 2).reshape(128, 24 * BC))


def pack_x_high(x_core):
    """4x skew-replicated, channel-summed high-branch input with the zero
    margins baked in: per tile a contiguous [128, 800] slab laid out as
    xh4[dv*32+i, t*800 + 8 + 48*b + jj] = sum_c x[t*16+b, c, i, jj+dv-3]
    for jj in [0,36), zeros elsewhere.  DMA'd contiguously on device.
    """
    xs = np.asarray(x_core, np.float32).sum(axis=1).astype(BF16NP)  # [BC,32,32]
    xpad = np.zeros((BC, 32, 42), BF16NP)
    xpad[:, :, 3:35] = xs
    arr = np.stack([xpad[:, :, dv:dv + 36] for dv in range(4)])
    arr = arr.reshape(4, NTILES, TIMGS, 32, 36).transpose(0, 3, 1, 2, 4)
    blocks = np.zeros((128, NTILES, TIMGS, 48), BF16NP)
    blocks[:, :, :, 0:36] = arr.reshape(128, NTILES, TIMGS, 36)
    full = np.zeros((128, NTILES, 800), BF16NP)
    full[:, :, 8:8 + 768] = blocks.reshape(128, NTILES, 768)
    return np.ascontiguousarray(full.reshape(128, NTILES * 800))


# ---------------------------------------------------------------- bass build
_WAIT_CARRIERS = ("InstEventSemaphore", "InstNoOp",
                  "InstUnconditionalBranch", "InstCompareAndBranch")


def _legalize_waits(nc):
    """Split excess semaphore waits onto same-engine NoOp carriers.

    The walrus codegen used by the bass2jax path allows at most 1 attached
    wait on compute instructions and 2 on DMA; Tile sometimes emits more.
    Engines execute instructions in order, so a preceding NoOp carrying the
    extra waits is equivalent.
    """
    uid = 0
    for blk in nc.m.functions[0].blocks:
        insts = blk.instructions
        i = 0
        while i < len(insts):
            inst = insts[i]
            ty = type(inst).__name__
            si = inst.sync_info
            if si is None or ty in _WAIT_CARRIERS:
                i += 1
                continue
            waits = list(si.on_wait or [])
            limit = 1
            if len(waits) <= limit:
                i += 1
                continue
            extra, keep = waits[:-limit], waits[-limit:]
            for w in extra:
                nop = mybir.InstNoOp(
                    name=f"waitnop-{uid}", engine=inst.engine,
                    sync_info=mybir.SyncInfo(on_wait=[w], on_update=[]))
                uid += 1
                insts.insert(i, nop)
                i += 1
            inst.sync_info = mybir.SyncInfo(
                on_wait=keep, on_update=list(si.on_update or []))
            i += 1


def build_nc(skip_conv=False, skip_low=False, skip_fc1=False, skip_fc23=False):
    nc = bass.Bass()
    xth_d = nc.declare_dram_parameter("xth", [128, NTILES * 800], BF16, isOutput=False)
    xl_d = nc.declare_dram_parameter("xl", [128, 24 * BC], BF16, isOutput=False)
    wc_d = nc.declare_dram_parameter("wc", [128, 2048], BF16, isOutput=False)
    wlow_d = nc.declare_dram_parameter("wlow", [128, 24 * 256], BF16, isOutput=False)
    w1_d = nc.declare_dram_parameter("w1r", [33, 128, 1024], BF16, isOutput=False)
    w2_d = nc.declare_dram_parameter("w2r", [128, 2048], BF16, isOutput=False)
    w3_d = nc.declare_dram_parameter("w3r", [128, 10], BF16, isOutput=False)
    b1_d = nc.declare_dram_parameter("b1r", [128, 8], FP32, isOutput=False)
    b2_d = nc.declare_dram_parameter("b2r", [128, 2], FP32, isOutput=False)
    b3_d = nc.declare_dram_parameter("b3r", [5, 1], FP32, isOutput=False)
    bl_d = nc.declare_dram_parameter("blowr", [64, 1], FP32, isOutput=False)
    y_d = nc.declare_dram_parameter("y", [5, 512], FP32, isOutput=True)

    RELU = mybir.ActivationFunctionType.Relu
    MAX = AluOpType.max

    with tile.TileContext(nc) as tc:
        with (
            tc.tile_pool(name="persist", bufs=1) as pp,
            tc.tile_pool(name="work", bufs=3) as wp,
            tc.tile_pool(name="w1pool", bufs=4) as w1p,
        ):
            # conv weights first: the first matmul depends only on these.
            # Everything else is DMA'd from inside the conv loop (below) so
            # the early skew-tile DMAs get the HBM bandwidth to themselves.
            wc_sb = pp.tile([128, 2048], BF16, tag="wc")
            nc.sync.dma_start(out=wc_sb[:], in_=wc_d[:])
            xl_sb = pp.tile([128, 24 * BC], BF16, tag="xl")
            wlow_sb = pp.tile([128, 24 * 256], BF16, tag="wlow")
            w2_sb = pp.tile([128, 2048], BF16, tag="w2")
            w3_sb = pp.tile([128, 10], BF16, tag="w3")
            b1_sb = pp.tile([128, 8], FP32, tag="b1")
            b2_sb = pp.tile([128, 2], FP32, tag="b2")
            b3_sb = pp.tile([5, 1], FP32, tag="b3")
            bl_sb = pp.tile([64, 1], FP32, tag="bl")

            # pooled high features, j2-major: ph[p, j2*512 + t*16 + b]
            ph0 = pp.tile([128, 8192], BF16, tag="ph0")
            ph1 = pp.tile([128, 8192], BF16, tag="ph1")
            xlow_sb = pp.tile([128, 512], BF16, tag="xlow")
            nc.gpsimd.memset(xlow_sb[:], 0.0)
            h1_sb = [pp.tile([128, 512], BF16, tag=f"h1_{i}", name=f"h1_{i}")
                     for i in range(8)]
            h2_sb = [pp.tile([128, 512], BF16, tag=f"h2_{i}", name=f"h2_{i}")
                     for i in range(2)]
            y_sb = pp.tile([5, 512], FP32, tag="ysb")

            ph0v = ph0[:].rearrange("p (j t b) -> p j t b", j=16, t=NTILES, b=TIMGS)
            ph1v = ph1[:].rearrange("p (j t b) -> p j t b", j=16, t=NTILES, b=TIMGS)

            # ---------------- conv + pool over 32 tiles
            with tc.tile_pool(name="cpsum", bufs=8, space="PSUM") as cps:
                deferred = {
                    6: lambda: nc.sync.dma_start(
                        out=xl_sb[:, 0:4096], in_=xl_d[:, 0:4096]),
                    8: lambda: nc.sync.dma_start(
                        out=xl_sb[:, 4096:8192], in_=xl_d[:, 4096:8192]),
                    10: lambda: nc.sync.dma_start(
                        out=xl_sb[:, 8192:12288], in_=xl_d[:, 8192:12288]),
                    12: lambda: nc.sync.dma_start(out=wlow_sb[:], in_=wlow_d[:]),
                    14: lambda: nc.sync.dma_start(out=w2_sb[:], in_=w2_d[:]),
                    16: lambda: [
                        nc.sync.dma_start(out=w3_sb[:], in_=w3_d[:]),
                        nc.sync.dma_start(out=b1_sb[:], in_=b1_d[:]),
                        nc.sync.dma_start(out=b2_sb[:], in_=b2_d[:]),
                        nc.sync.dma_start(out=b3_sb[:], in_=b3_d[:]),
                        nc.sync.dma_start(out=bl_sb[:], in_=bl_d[:]),
                    ],
                }
                for t in range(0 if skip_conv else NTILES):
                    if t in deferred:
                        deferred.pop(t)()
                    # contiguous DMA of the pre-padded skew tile
                    xs4 = wp.tile([128, 800], BF16, tag="xs4", bufs=4)
                    nc.gpsimd.dma_start(
                        out=xs4[:], in_=xth_d[:, t * 800:(t + 1) * 800])

                    cp = [cps.tile([128, 512], FP32, tag="cp", name=f"cp{t}_{i}") for i in range(4)]
                    for kc in range(4):
                        off = 4 * kc + 4
                        rhs = (
                            xs4[:, off:off + 768]
                            .rearrange("p (b j) -> p b j", j=48)[:, :, 0:32]
                        )
                        for mc in range(4):
                            nc.tensor.matmul(
                                cp[mc][:],
                                wc_sb[:, kc * 512 + mc * 128: kc * 512 + (mc + 1) * 128],
                                rhs,
                                start=(kc == 0),
                                stop=(kc == 3),
                            )
                    # relu on scalar (PSUM->SBUF, 1 read each), pool on vector
                    s = [wp.tile([128, 512], BF16, tag=f"s{i}", name=f"s{t}_{i}")
                         for i in range(4)]
                    for i in range(4):
                        nc.scalar.activation(out=s[i][:], in_=cp[i][:], func=RELU)
                    m0 = wp.tile([128, 512], BF16, tag="m0")
                    m1 = wp.tile([128, 512], BF16, tag="m1")
                    nc.vector.tensor_max(out=m0[:], in0=s[0][:], in1=s[2][:])
                    nc.vector.tensor_max(out=m1[:], in0=s[1][:], in1=s[3][:])
                    m0v = m0[:].rearrange("p (b j t) -> p j b t", j=16, t=2)
                    m1v = m1[:].rearrange("p (b j t) -> p j b t", j=16, t=2)
                    nc.vector.tensor_max(
                        out=ph0v[:, :, t, :], in0=m0v[:, :, :, 0], in1=m0v[:, :, :, 1])
                    nc.vector.tensor_max(
                        out=ph1v[:, :, t, :], in0=m1v[:, :, :, 0], in1=m1v[:, :, :, 1])
                for fn in deferred.values():
                    fn()

            # ---------------- low branch
            with tc.tile_pool(name="lpsum", bufs=1, space="PSUM") as lps:
                lpA = lps.tile([128, 512], FP32, tag="lpA")
                lpB = lps.tile([128, 512], FP32, tag="lpB")
                for kc in range(0 if skip_low else 24):
                    rhs = xl_sb[:, kc * 512:(kc + 1) * 512]
                    nc.tensor.matmul(
                        lpA[:], wlow_sb[:, kc * 256:kc * 256 + 128], rhs,
                        start=(kc == 0), stop=(kc == 23))
                    nc.tensor.matmul(
                        lpB[:], wlow_sb[:, kc * 256 + 128:(kc + 1) * 256], rhs,
                        start=(kc == 0), stop=(kc == 23))
                sB = wp.tile([128, 512], BF16, tag="sB")
                nc.scalar.activation(
                    out=sB[:], in_=lpB[:],
                    func=mybir.ActivationFunctionType.Copy)
                mAB = wp.tile([128, 512], BF16, tag="mAB")
                nc.vector.tensor_max(out=mAB[:], in0=lpA[:], in1=sB[:])
                # partition shift via sbuf->sbuf DMA, then final pool max
                tmp = wp.tile([64, 512], BF16, tag="ltmp")
                nc.sync.dma_start(out=tmp[:], in_=mAB[64:128, :])
                mm64 = wp.tile([64, 512], BF16, tag="mm64")
                nc.vector.tensor_max(out=mm64[:], in0=mAB[0:64, :], in1=tmp[:])
                nc.scalar.activation(
                    out=xlow_sb[0:64, :], in_=mm64[:], func=RELU,
                    bias=bl_sb[:, 0:1])

            # ---------------- FC1 (weights streamed)
            with tc.tile_pool(name="fpsum", bufs=1, space="PSUM") as fps:
                fp = [fps.tile([128, 512], FP32, tag=f"fp{i}", name=f"fp{i}") for i in range(8)]
                for ks in range(0 if skip_fc1 else 33):
                    w1t = w1p.tile([128, 1024], BF16, tag="w1t", bufs=8)
                    nc.gpsimd.dma_start(out=w1t[:], in_=w1_d[ks])
                    if ks < 32:
                        j2, c = ks // 2, ks % 2
                        src = ph0 if c == 0 else ph1
                        rhs = src[:, j2 * 512:(j2 + 1) * 512]
                    else:
                        rhs = xlow_sb[:]
                    for mc in range(8):
                        nc.tensor.matmul(
                            fp[mc][:], w1t[:, mc * 128:(mc + 1) * 128], rhs,
                            start=(ks == 0), stop=(ks == 32))
                # alternate engines so the 8 bias+relu's don't serialize
                for mc in range(8):
                    if mc % 2 == 0:
                        nc.scalar.activation(
                            out=h1_sb[mc][:], in_=fp[mc][:],
                            func=RELU, bias=b1_sb[:, mc:mc + 1])
                    else:
                        nc.vector.tensor_scalar(
                            out=h1_sb[mc][:], in0=fp[mc][:],
                            scalar1=b1_sb[:, mc:mc + 1], scalar2=0.0,
                            op0=AluOpType.add, op1=AluOpType.max)

            # ---------------- FC2 + FC3
            with tc.tile_pool(name="gpsum", bufs=1, space="PSUM") as gps:
                gp = [gps.tile([128, 512], FP32, tag=f"gp{i}", name=f"gp{i}") for i in range(2)]
                for kc in range(0 if skip_fc23 else 8):
                    rhs = h1_sb[kc][:]
                    for mc in range(2):
                        nc.tensor.matmul(
                            gp[mc][:],
                            w2_sb[:, kc * 256 + mc * 128: kc * 256 + (mc + 1) * 128],
                            rhs, start=(kc == 0), stop=(kc == 7))
                nc.scalar.activation(
                    out=h2_sb[0][:], in_=gp[0][:], func=RELU,
                    bias=b2_sb[:, 0:1])
                nc.vector.tensor_scalar(
                    out=h2_sb[1][:], in0=gp[1][:], scalar1=b2_sb[:, 1:2],
                    scalar2=0.0, op0=AluOpType.add, op1=AluOpType.max)

                yp = gps.tile([128, 512], FP32, tag="yp")
                for kc in range(2):
                    nc.tensor.matmul(
                        yp[0:5, :], w3_sb[:, kc * 5:(kc + 1) * 5],
                        h2_sb[kc][:],
                        start=(kc == 0), stop=(kc == 1))
                nc.vector.tensor_scalar_add(
                    out=y_sb[:], in0=yp[0:5, :], scalar1=b3_sb[:, 0:1])
                nc.sync.dma_start(out=y_d[:], in_=y_sb[:])

    _legalize_waits(nc)
    return nc


_NC = None
TRACE = False
TRACE_DIR = None
LAST_RESULT = None


def kernel(**inputs):
    global _NC, LAST_RESULT
    w = build_weights(inputs)
    if _NC is None:
        _NC = build_nc()
    x = np.asarray(inputs["x"], np.float32)
    in_maps = []
    for c in range(NCORES):
        xs_c = x[c * BC:(c + 1) * BC]
        m = {"xl": pack_x_low(xs_c), "xth": pack_x_high(xs_c)}
        m.update(w)
        in_maps.append(m)
    res = run_bass_kernel_spmd(_NC, in_maps, list(range(NCORES)), trace=TRACE,
                               tmpdir=TRACE_DIR)
    LAST_RESULT = res
    y = np.concatenate(
        [np.asarray(res.results[i]["y"], np.float32).T for i in range(NCORES)], axis=0)
    return y


# revision 34
# speedup vs baseline: 1.2134x; 1.0308x over previous
"""Trainium2 Bass kernel for nn_MixedResolutionCNN.

Network (per sample, eval mode):
  high branch: ridgelet conv 3->16 k=15 same-pad (kernel broadcast over in-ch)
               -> relu -> maxpool2 -> 4096 feats
  low branch:  bilinear resize 32->8 -> conv 3->4 k=3 pad1 + bias -> relu
               -> maxpool2 -> 64 feats
  head:        concat -> fc 4160->1024 relu -> 1024->256 relu -> 256->5

Device strategy (pure data parallel over 8 cores, 512 images/core):

* The ridgelet kernel is identical across the 3 input channels, so the high
  conv contracts the channel-summed image xs = sum_c x[:,c] with a 16x15x15
  kernel. Expressed as matmuls with contraction over (v, i') = (kernel col,
  image row): out[(o,i),(b,j)] = sum Khat[o, i'-i+7, v] * xs[b, i', j+v-7].
  The moving operand for v-chunk kc is a skewed 4x replication of the
  column-padded image rows: block dv holds xs shifted by dv columns so a
  single strided AP reads xs[b, i', j + (4kc+dv) - 7] for all 128
  partitions.  4 K-chunks x 4 M-chunks of [128,128,512] matmuls per
  16-image tile.
* relu/maxpool fold into the pool maxes (relu(max(a,b)) == max(0,a,b) via
  one scalar_tensor_tensor op on the vector engine); pooled features are
  written j2-major so every FC1 rhs chunk is a fully contiguous [128,512]
  slab (peak-rate matmul feed).  Channel-sum adds run on gpsimd to keep the
  vector engine under the tensor-engine roofline.
* low branch: resize+conv fold into one linear map [3072, 256]; x is packed
  host-side as [feature, batch] so the 24 K-chunks stream contiguously.
  Output partitions hold the 4 pool-parity groups (2 matmuls of 128).
* FC1 weights (8.5MB bf16) stream from DRAM, double buffered; everything
  else is resident in SBUF.  Conv weights are DMA'd first so the tensor
  engine starts within ~2us.
"""

import numpy as np
import ml_dtypes

import concourse.bass as bass
import concourse.tile as tile
from concourse import mybir
from concourse.alu_op_type import AluOpType
from concourse.bass_utils import run_bass_kernel_spmd

BF16NP = ml_dtypes.bfloat16
FP32 = mybir.dt.float32
BF16 = mybir.dt.bfloat16

B = 4096
NCORES = 8
BC = B // NCORES           # 512 images per core
TIMGS = 16                 # images per tile
NTILES = BC // TIMGS       # 32
KS = 15
OUT_CH = 16


# ---------------------------------------------------------------- host math
def _ridgelet_kernel(r_dirs, r_scales, r_pos):
    """[16,15,15] channel-shared ridgelet kernel, mirrors reference."""
    c = np.arange(KS, dtype=np.float32) - KS // 2
    x1 = c[:, None]
    x2 = c[None, :]
    d = np.asarray(r_dirs, np.float32)[:, None, None]
    s = np.asarray(r_scales, np.float32)[:, None, None]
    p = np.asarray(r_pos, np.float32)[:, None, None]
    t = (x1 * np.cos(d) + x2 * np.sin(d) - p) / s
    vals = np.exp(-t * t / 2.0) - 0.5 * np.exp(-t * t / 8.0)
    return vals.reshape(OUT_CH, 10, KS, KS).sum(axis=1)


def _resize_mat(in_size=32, out_size=8):
    """Row matrix of jax.image.resize(..., 'bilinear', antialias=True)."""
    scale = out_size / in_size
    inv = 1.0 / scale
    kscale = max(inv, 1.0)
    sample_f = (np.arange(out_size, dtype=np.float64) + 0.5) * inv - 0.5
    x = np.abs(sample_f[None, :] - np.arange(in_size, dtype=np.float64)[:, None])
    w = np.maximum(0.0, 1.0 - x / kscale)
    w = w / w.sum(axis=0, keepdims=True)
    return w.T.astype(np.float32)  # [out, in]


def build_weights(inputs):
    """All packed device arrays (shared across cores)."""
    khat = _ridgelet_kernel(inputs["r_dirs"], inputs["r_scales"], inputs["r_pos"])
    # padded to 16x16 so v=15 / u out-of-range index to a zero slot
    khat_p = np.zeros((OUT_CH, 16, 16), np.float32)
    khat_p[:, :KS, :KS] = khat

    # conv lhsT: wc[p=(dv,i'), kc*512 + ch*128 + wi]
    dvip = np.arange(128)
    dv = dvip // 32
    ip = dvip % 32
    m = np.arange(512)
    ch = m // 128
    wi = m % 128
    par = ch // 2          # i parity (0=even rows, 1=odd)
    oh = ch % 2            # o half
    o = oh * 8 + wi // 16
    i2 = wi % 16
    i = 2 * i2 + par
    wc = np.zeros((128, 2048), np.float32)
    u = ip[:, None] - i[None, :] + 7          # [128, 512]
    umask = (u >= 0) & (u < KS)
    uc = np.clip(u, 0, 15)
    for kc in range(4):
        v = 4 * kc + dv                        # [128]
        vals = khat_p[o[None, :], uc, np.clip(v, 0, 15)[:, None]]
        vals = np.where(umask, vals, 0.0)
        wc[:, kc * 512:(kc + 1) * 512] = vals

    # low branch: fold resize+conv into [3072, 256]
    A = _resize_mat()
    Ash = np.zeros((3, 8, 32), np.float32)
    for dh in range(3):
        for ph in range(8):
            r = ph + dh - 1
            if 0 <= r < 8:
                Ash[dh, ph] = A[r]
    wlow = np.asarray(inputs["wlow"], np.float32)
    # D[c,i,w,o,ph,pw] = sum_{dh,dw} wlow[o,c,dh,dw] Ash[dh,ph,i] Ash[dw,pw,w]
    D = np.einsum("ocuv,upi,vqw->ciwopq", wlow, Ash, Ash).astype(np.float32)
    Dp = D.reshape(3072, 4, 8, 8)              # [(c,i,w), o, ph, pw]
    # out col layout: 2 matmuls of 128.  Pool partners sit at the SAME
    # partition in the two PSUM banks (A holds groups 0,2; B holds 1,3) so
    # the first pool max never crosses partitions.
    Wn = np.zeros((3072, 2, 128), np.float32)
    G = [(0, 0), (0, 1), (1, 0), (1, 1)]
    for g, (pp_, qq) in enumerate(G):
        blk = Dp[:, :, pp_::2, qq::2].reshape(3072, 64)
        Wn[:, g % 2, (g // 2) * 64:(g // 2) * 64 + 64] = blk
    wlowp = np.ascontiguousarray(
        Wn.reshape(24, 128, 256).transpose(1, 0, 2).reshape(128, 24 * 256))

    # FC1 reorder: kstep = j2*2 + chunk over high feats, kstep 32 = low
    w1 = np.asarray(inputs["w1"], np.float32)          # [1024, 4160]
    w1hi = w1[:, 64:].reshape(1024, 16, 16, 16)        # [n, o, i2, j2]
    w1r = np.zeros((33, 128, 1024), np.float32)
    for ks in range(32):
        j2, c = ks // 2, ks % 2
        blk = w1hi[:, 8 * c:8 * (c + 1), :, j2]        # [n, 8, 16]
        w1r[ks] = blk.reshape(1024, 128).T
    w1r[32, :64, :] = w1[:, :64].T

    w2 = np.asarray(inputs["w2"], np.float32)          # [256, 1024]
    w2r = np.zeros((128, 2048), np.float32)
    for kc in range(8):
        w2r[:, kc * 256:(kc + 1) * 256] = w2[:, kc * 128:(kc + 1) * 128].T
    w3 = np.asarray(inputs["w3"], np.float32)          # [5, 256]
    w3r = np.zeros((128, 10), np.float32)
    for kc in range(2):
        w3r[:, kc * 5:(kc + 1) * 5] = w3[:, kc * 128:(kc + 1) * 128].T

    b1r = np.asarray(inputs["b1"], np.float32).reshape(8, 128).T.copy()
    b2r = np.asarray(inputs["b2"], np.float32).reshape(2, 128).T.copy()
    b3r = np.asarray(inputs["b3"], np.float32)[:, None].copy()
    blowr = np.repeat(np.asarray(inputs["blow"], np.float32), 16)[:, None].copy()

    return {
        "wc": wc.astype(BF16NP),
        "wlow": wlowp.astype(BF16NP),
        "w1r": w1r.astype(BF16NP),
        "w2r": w2r.astype(BF16NP),
        "w3r": w3r.astype(BF16NP),
        "b1r": np.ascontiguousarray(b1r),
        "b2r": np.ascontiguousarray(b2r),
        "b3r": b3r,
        "blowr": np.ascontiguousarray(blowr),
    }


def pack_x_low(x_core):
    """[512,3,32,32] f32 -> [128, 24*512] bf16: xl[p, kc*512+tb] =
    x[tb, (kc*128+p)//1024, ...] i.e. feature-major transpose."""
    xc = np.asarray(x_core, np.float32).astype(BF16NP)
    arr = xc.transpose(1, 2, 3, 0).reshape(3072, BC)      # [(c,i,w), tb]
    return np.ascontiguousarray(
        arr.reshape(24, 128, BC).transpose(1, 0, 2).reshape(128, 24 * BC))


def pack_x_high(x_core):
    """4x skew-replicated, channel-summed high-branch input with the zero
    margins baked in: per tile a contiguous [128, 800] slab laid out as
    xh4[dv*32+i, t*800 + 8 + 48*b + jj] = sum_c x[t*16+b, c, i, jj+dv-3]
    for jj in [0,36), zeros elsewhere.  DMA'd contiguously on device.
    """
    xs = np.asarray(x_core, np.float32).sum(axis=1).astype(BF16NP)  # [BC,32,32]
    xpad = np.zeros((BC, 32, 42), BF16NP)
    xpad[:, :, 3:35] = xs
    arr = np.stack([xpad[:, :, dv:dv + 36] for dv in range(4)])
    arr = arr.reshape(4, NTILES, TIMGS, 32, 36).transpose(0, 3, 1, 2, 4)
    blocks = np.zeros((128, NTILES, TIMGS, 48), BF16NP)
    blocks[:, :, :, 0:36] = arr.reshape(128, NTILES, TIMGS, 36)
    full = np.zeros((128, NTILES, 800), BF16NP)
    full[:, :, 8:8 + 768] = blocks.reshape(128, NTILES, 768)
    return np.ascontiguousarray(full.reshape(128, NTILES * 800))


# ---------------------------------------------------------------- bass build
_WAIT_CARRIERS = ("InstEventSemaphore", "InstNoOp",
                  "InstUnconditionalBranch", "InstCompareAndBranch")


def _legalize_waits(nc):
    """Split excess semaphore waits onto same-engine NoOp carriers.

    The walrus codegen used by the bass2jax path allows at most 1 attached
    wait on compute instructions and 2 on DMA; Tile sometimes emits more.
    Engines execute instructions in order, so a preceding NoOp carrying the
    extra waits is equivalent.
    """
    uid = 0
    for blk in nc.m.functions[0].blocks:
        insts = blk.instructions
        i = 0
        while i < len(insts):
            inst = insts[i]
            ty = type(inst).__name__
            si = inst.sync_info
            if si is None or ty in _WAIT_CARRIERS:
                i += 1
                continue
            waits = list(si.on_wait or [])
            limit = 1
            if len(waits) <= limit:
                i += 1
                continue
            extra, keep = waits[:-limit], waits[-limit:]
            for w in extra:
                nop = mybir.InstNoOp(
                    name=f"waitnop-{uid}", engine=inst.engine,
                    sync_info=mybir.SyncInfo(on_wait=[w], on_update=[]))
                uid += 1
                insts.insert(i, nop)
                i += 1
            inst.sync_info = mybir.SyncInfo(
                on_wait=keep, on_update=list(si.on_update or []))
            i += 1


def build_nc(skip_conv=False, skip_low=False, skip_fc1=False, skip_fc23=False):
    nc = bass.Bass()
    xth_d = nc.declare_dram_parameter("xth", [128, NTILES * 800], BF16, isOutput=False)
    xl_d = nc.declare_dram_parameter("xl", [128, 24 * BC], BF16, isOutput=False)
    wc_d = nc.declare_dram_parameter("wc", [128, 2048], BF16, isOutput=False)
    wlow_d = nc.declare_dram_parameter("wlow", [128, 24 * 256], BF16, isOutput=False)
    w1_d = nc.declare_dram_parameter("w1r", [33, 128, 1024], BF16, isOutput=False)
    w2_d = nc.declare_dram_parameter("w2r", [128, 2048], BF16, isOutput=False)
    w3_d = nc.declare_dram_parameter("w3r", [128, 10], BF16, isOutput=False)
    b1_d = nc.declare_dram_parameter("b1r", [128, 8], FP32, isOutput=False)
    b2_d = nc.declare_dram_parameter("b2r", [128, 2], FP32, isOutput=False)
    b3_d = nc.declare_dram_parameter("b3r", [5, 1], FP32, isOutput=False)
    bl_d = nc.declare_dram_parameter("blowr", [64, 1], FP32, isOutput=False)
    y_d = nc.declare_dram_parameter("y", [5, 512], FP32, isOutput=True)

    RELU = mybir.ActivationFunctionType.Relu
    MAX = AluOpType.max

    with tile.TileContext(nc) as tc:
        with (
            tc.tile_pool(name="persist", bufs=1) as pp,
            tc.tile_pool(name="work", bufs=3) as wp,
            tc.tile_pool(name="w1pool", bufs=4) as w1p,
        ):
            # conv weights first: the first matmul depends only on these.
            # Everything else is DMA'd from inside the conv loop (below) so
            # the early skew-tile DMAs get the HBM bandwidth to themselves.
            wc_sb = pp.tile([128, 2048], BF16, tag="wc")
            nc.sync.dma_start(out=wc_sb[:], in_=wc_d[:])
            xl_sb = pp.tile([128, 24 * BC], BF16, tag="xl")
            wlow_sb = pp.tile([128, 24 * 256], BF16, tag="wlow")
            w2_sb = pp.tile([128, 2048], BF16, tag="w2")
            w3_sb = pp.tile([128, 10], BF16, tag="w3")
            b1_sb = pp.tile([128, 8], FP32, tag="b1")
            b2_sb = pp.tile([128, 2], FP32, tag="b2")
            b3_sb = pp.tile([5, 1], FP32, tag="b3")
            bl_sb = pp.tile([64, 1], FP32, tag="bl")

            # pooled high features, j2-major: ph[p, j2*512 + t*16 + b]
            ph0 = pp.tile([128, 8192], BF16, tag="ph0")
            ph1 = pp.tile([128, 8192], BF16, tag="ph1")
            xlow_sb = pp.tile([128, 512], BF16, tag="xlow")
            nc.gpsimd.memset(xlow_sb[:], 0.0)
            h1_sb = [pp.tile([128, 512], BF16, tag=f"h1_{i}", name=f"h1_{i}")
                     for i in range(8)]
            h2_sb = [pp.tile([128, 512], BF16, tag=f"h2_{i}", name=f"h2_{i}")
                     for i in range(2)]
            y_sb = pp.tile([5, 512], FP32, tag="ysb")

            ph0v = ph0[:].rearrange("p (j t b) -> p j t b", j=16, t=NTILES, b=TIMGS)
            ph1v = ph1[:].rearrange("p (j t b) -> p j t b", j=16, t=NTILES, b=TIMGS)

            # ---------------- conv + pool over 32 tiles
            with tc.tile_pool(name="cpsum", bufs=8, space="PSUM") as cps:
                # big preloads issued from the SCALAR engine's queue:
                # scalar only reaches these between its per-tile relu ops,
                # so the transfers are time-gated behind conv progress and
                # don't steal HBM from the skew-tile stream.
                deferred = {
                    6: lambda: nc.scalar.dma_start(
                        out=xl_sb[:, 0:4096], in_=xl_d[:, 0:4096]),
                    8: lambda: nc.scalar.dma_start(
                        out=xl_sb[:, 4096:8192], in_=xl_d[:, 4096:8192]),
                    10: lambda: nc.scalar.dma_start(
                        out=xl_sb[:, 8192:12288], in_=xl_d[:, 8192:12288]),
                    12: lambda: nc.scalar.dma_start(out=wlow_sb[:], in_=wlow_d[:]),
                    14: lambda: nc.scalar.dma_start(out=w2_sb[:], in_=w2_d[:]),
                    16: lambda: [
                        nc.scalar.dma_start(out=w3_sb[:], in_=w3_d[:]),
                        nc.scalar.dma_start(out=b1_sb[:], in_=b1_d[:]),
                        nc.scalar.dma_start(out=b2_sb[:], in_=b2_d[:]),
                        nc.scalar.dma_start(out=b3_sb[:], in_=b3_d[:]),
                        nc.scalar.dma_start(out=bl_sb[:], in_=bl_d[:]),
                    ],
                }
                for t in range(0 if skip_conv else NTILES):
                    if t in deferred:
                        deferred.pop(t)()
                    # contiguous DMA of the pre-padded skew tile
                    xs4 = wp.tile([128, 800], BF16, tag="xs4", bufs=6)
                    nc.gpsimd.dma_start(
                        out=xs4[:], in_=xth_d[:, t * 800:(t + 1) * 800])

                    cp = [cps.tile([128, 512], FP32, tag="cp", name=f"cp{t}_{i}") for i in range(4)]
                    for kc in range(4):
                        off = 4 * kc + 4
                        rhs = (
                            xs4[:, off:off + 768]
                            .rearrange("p (b j) -> p b j", j=48)[:, :, 0:32]
                        )
                        for mc in range(4):
                            nc.tensor.matmul(
                                cp[mc][:],
                                wc_sb[:, kc * 512 + mc * 128: kc * 512 + (mc + 1) * 128],
                                rhs,
                                start=(kc == 0),
                                stop=(kc == 3),
                            )
                    # relu on scalar (PSUM->SBUF, 1 read each), pool on vector
                    s = [wp.tile([128, 512], BF16, tag=f"s{i}", name=f"s{t}_{i}")
                         for i in range(4)]
                    for i in range(4):
                        nc.scalar.activation(out=s[i][:], in_=cp[i][:], func=RELU)
                    m0 = wp.tile([128, 512], BF16, tag="m0")
                    m1 = wp.tile([128, 512], BF16, tag="m1")
                    nc.vector.tensor_max(out=m0[:], in0=s[0][:], in1=s[2][:])
                    nc.vector.tensor_max(out=m1[:], in0=s[1][:], in1=s[3][:])
                    m0v = m0[:].rearrange("p (b j t) -> p j b t", j=16, t=2)
                    m1v = m1[:].rearrange("p (b j t) -> p j b t", j=16, t=2)
                    nc.vector.tensor_max(
                        out=ph0v[:, :, t, :], in0=m0v[:, :, :, 0], in1=m0v[:, :, :, 1])
                    nc.vector.tensor_max(
                        out=ph1v[:, :, t, :], in0=m1v[:, :, :, 0], in1=m1v[:, :, :, 1])
                for fn in deferred.values():
                    fn()

            # ---------------- low branch
            with tc.tile_pool(name="lpsum", bufs=1, space="PSUM") as lps:
                lpA = lps.tile([128, 512], FP32, tag="lpA")
                lpB = lps.tile([128, 512], FP32, tag="lpB")
                for kc in range(0 if skip_low else 24):
                    rhs = xl_sb[:, kc * 512:(kc + 1) * 512]
                    nc.tensor.matmul(
                        lpA[:], wlow_sb[:, kc * 256:kc * 256 + 128], rhs,
                        start=(kc == 0), stop=(kc == 23))
                    nc.tensor.matmul(
                        lpB[:], wlow_sb[:, kc * 256 + 128:(kc + 1) * 256], rhs,
                        start=(kc == 0), stop=(kc == 23))
                sB = wp.tile([128, 512], BF16, tag="sB")
                nc.scalar.activation(
                    out=sB[:], in_=lpB[:],
                    func=mybir.ActivationFunctionType.Copy)
                mAB = wp.tile([128, 512], BF16, tag="mAB")
                nc.vector.tensor_max(out=mAB[:], in0=lpA[:], in1=sB[:])
                # partition shift via sbuf->sbuf DMA, then final pool max
                tmp = wp.tile([64, 512], BF16, tag="ltmp")
                nc.sync.dma_start(out=tmp[:], in_=mAB[64:128, :])
                mm64 = wp.tile([64, 512], BF16, tag="mm64")
                nc.vector.tensor_max(out=mm64[:], in0=mAB[0:64, :], in1=tmp[:])
                nc.scalar.activation(
                    out=xlow_sb[0:64, :], in_=mm64[:], func=RELU,
                    bias=bl_sb[:, 0:1])

            # ---------------- FC1 (weights streamed)
            with tc.tile_pool(name="fpsum", bufs=1, space="PSUM") as fps:
                fp = [fps.tile([128, 512], FP32, tag=f"fp{i}", name=f"fp{i}") for i in range(8)]
                for ks in range(0 if skip_fc1 else 33):
                    w1t = w1p.tile([128, 1024], BF16, tag="w1t", bufs=8)
                    nc.gpsimd.dma_start(out=w1t[:], in_=w1_d[ks])
                    if ks < 32:
                        j2, c = ks // 2, ks % 2
                        src = ph0 if c == 0 else ph1
                        rhs = src[:, j2 * 512:(j2 + 1) * 512]
                    else:
                        rhs = xlow_sb[:]
                    for mc in range(8):
                        nc.tensor.matmul(
                            fp[mc][:], w1t[:, mc * 128:(mc + 1) * 128], rhs,
                            start=(ks == 0), stop=(ks == 32))
                # alternate engines so the 8 bias+relu's don't serialize
                for mc in range(8):
                    if mc % 2 == 0:
                        nc.scalar.activation(
                            out=h1_sb[mc][:], in_=fp[mc][:],
                            func=RELU, bias=b1_sb[:, mc:mc + 1])
                    else:
                        nc.vector.tensor_scalar(
                            out=h1_sb[mc][:], in0=fp[mc][:],
                            scalar1=b1_sb[:, mc:mc + 1], scalar2=0.0,
                            op0=AluOpType.add, op1=AluOpType.max)

            # ---------------- FC2 + FC3
            with tc.tile_pool(name="gpsum", bufs=1, space="PSUM") as gps:
                gp = [gps.tile([128, 512], FP32, tag=f"gp{i}", name=f"gp{i}") for i in range(2)]
                for kc in range(0 if skip_fc23 else 8):
                    rhs = h1_sb[kc][:]
                    for mc in range(2):
                        nc.tensor.matmul(
                            gp[mc][:],
                            w2_sb[:, kc * 256 + mc * 128: kc * 256 + (mc + 1) * 128],
                            rhs, start=(kc == 0), stop=(kc == 7))
                nc.scalar.activation(
                    out=h2_sb[0][:], in_=gp[0][:], func=RELU,
                    bias=b2_sb[:, 0:1])
                nc.vector.tensor_scalar(
                    out=h2_sb[1][:], in0=gp[1][:], scalar1=b2_sb[:, 1:2],
                    scalar2=0.0, op0=AluOpType.add, op1=AluOpType.max)

                yp = gps.tile([128, 512], FP32, tag="yp")
                for kc in range(2):
                    nc.tensor.matmul(
                        yp[0:5, :], w3_sb[:, kc * 5:(kc + 1) * 5],
                        h2_sb[kc][:],
                        start=(kc == 0), stop=(kc == 1))
                nc.vector.tensor_scalar_add(
                    out=y_sb[:], in0=yp[0:5, :], scalar1=b3_sb[:, 0:1])
                nc.sync.dma_start(out=y_d[:], in_=y_sb[:])

    _legalize_waits(nc)
    return nc


_NC = None
TRACE = False
TRACE_DIR = None
LAST_RESULT = None


def kernel(**inputs):
    global _NC, LAST_RESULT
    w = build_weights(inputs)
    if _NC is None:
        _NC = build_nc()
    x = np.asarray(inputs["x"], np.float32)
    in_maps = []
    for c in range(NCORES):
        xs_c = x[c * BC:(c + 1) * BC]
        m = {"xl": pack_x_low(xs_c), "xth": pack_x_high(xs_c)}
        m.update(w)
        in_maps.append(m)
    res = run_bass_kernel_spmd(_NC, in_maps, list(range(NCORES)), trace=TRACE,
                               tmpdir=TRACE_DIR)
    LAST_RESULT = res
    y = np.concatenate(
        [np.asarray(res.results[i]["y"], np.float32).T for i in range(NCORES)], axis=0)
    return y
